# revision 9
# baseline (speedup 1.0000x reference)
"""MoE routing kernel for Trainium2 (8 NeuronCores).

Problem: out[b,l,:] = actions[b,l,:]                      if action_type[b,l] == 0
         out[b,l,:] = W[t-1] @ actions[b,l,:] + b[t-1]    if action_type == t >= 1

Strategy (type-parallel, bf16): route each token to the single expert it
needs. The host groups the B*L tokens by action_type; core t (t=1..7)
processes the tokens of expert t-1 as one dense [C, D] x [D, D] matmul in
bf16 (rel RMSE ~3e-3, well under the 2e-2 gate). Identity-type tokens are
copied on the host (exact); core 0 runs the same SPMD program on zero
inputs and its output is discarded.

Device schedule: 256-token tiles, each = 4 PSUM groups [128 tok, 512 feat]
accumulated over 8 contraction chunks; two alternating 4-bank PSUM sets so
tile i+1's matmuls never wait on tile i's PSUM evacuation. Host packs x/w
into SBUF-ready [128, free] bf16 blocks so DMAs are large and contiguous.
Weight DMAs dispatch on the sync queue while x DMAs dispatch on the
scalar (first tile) / gpsimd (steady) queues in parallel (dispatch is ~650ns each, serialized per queue); the
first w/x chunks are split fine (ic0 alone) so the first matmul starts
~2us after the fixed ~7us NEFF preamble. PSUM->SBUF bf16 casts are split
across the vector and scalar engines; outputs leave as one merged
[128, 2048] DMA per tile (partition-major DRAM layout, host unpacks).
"""

import sys

for _p in ("/root/.axon_site/_ro/trn_rl_repo", "/opt/trn_rl_repo"):
    if _p not in sys.path:
        sys.path.append(_p)

import numpy as np
import ml_dtypes
import concourse.bass as bass
import concourse.tile as tile
from concourse import bacc, mybir
from concourse.bass_utils import run_bass_kernel_spmd

D = 1024
P = 128
N_CORES = 8
TT = 256  # token tile
FB = 512  # psum feature block
NIC = D // P  # 8 contraction chunks
NOB = D // FB  # 2 output feature blocks
F32 = mybir.dt.float32
BF16 = mybir.dt.bfloat16
BF16NP = ml_dtypes.bfloat16

_program_cache: dict[tuple, bass.Bass] = {}


def _t_tiles(C):
    tiles = []
    t0 = 0
    while t0 < C:
        tt = min(TT, C - t0)
        tiles.append((t0, tt))
        t0 += tt
    return tiles


def build_program(C: int, with_bias: bool) -> bass.Bass:
    """out = x @ w.T per-core, x/w host-packed bf16.

    DRAM inputs per core:
      xP [P, 8*C]  : cols [(NIC*t0 + ic*tt) ...] hold
                     x.T[ic*128:(ic+1)*128, t0:t0+tt]  (contract chunk ic,
                     token tile [t0, t0+tt)) -- SBUF-ready, ic-major per tile.
      wP [P, 8*D]  : cols [ic*D ...] = w.T[ic*128:(ic+1)*128, :]
      bB [P, D]    : broadcast bias rows (only if with_bias)
    DRAM output: outP [P, C*D/P] bf16, partition-major: token t0+c*128+p,
    feature f lives at outP[p, (t0//128 + c)*D + f]. Host unpacks.
    """
    key = (C, with_bias)
    if key in _program_cache:
        return _program_cache[key]
    nc = bacc.Bacc("TRN2", target_bir_lowering=False, debug=False, num_devices=N_CORES)
    xP = nc.dram_tensor("xP", [P, NIC * C], BF16, kind="ExternalInput")
    wP = nc.dram_tensor("wP", [P, NIC * D], BF16, kind="ExternalInput")
    bB = nc.dram_tensor("bB", [P, D], F32, kind="ExternalInput") if with_bias else None
    outP = nc.dram_tensor("outP", [P, (C // P) * D], BF16, kind="ExternalOutput")

    tiles = _t_tiles(C)
    W_CHUNKS = [(0, 1), (1, 1), (2, 2), (4, 4)]
    X0_CHUNKS = [(0, 1), (1, 3), (4, 4)]

    with tile.TileContext(nc) as tc:
        with (
            tc.tile_pool(name="wpool", bufs=1) as wpool,
            tc.tile_pool(name="bpool", bufs=1) as bpool,
            tc.tile_pool(name="xpool", bufs=2) as xpool,
            tc.tile_pool(name="opool", bufs=2) as opool,
            tc.tile_pool(name="psum", bufs=1, space="PSUM") as psum_pool,
        ):
            tt0 = tiles[0][1]
            w_tiles = [None] * NIC  # per-ic (tile, col offset) views
            x0_tiles = [None] * NIC

            def _dma_w(ic0_, nic_):
                wt = wpool.tile([P, nic_ * D], BF16, name=f"w{ic0_}", tag=f"w{ic0_}")
                nc.sync.dma_start(wt[:], wP[:, ic0_ * D : (ic0_ + nic_) * D])
                for j in range(nic_):
                    w_tiles[ic0_ + j] = (wt, j * D)

            def _dma_x0(ic0_, nic_):
                xt = xpool.tile(
                    [P, nic_ * tt0], BF16, name=f"x0_{ic0_}", tag=f"x0_{ic0_}"
                )
                nc.scalar.dma_start(xt[:], xP[:, ic0_ * tt0 : (ic0_ + nic_) * tt0])
                for j in range(nic_):
                    x0_tiles[ic0_ + j] = (xt, j * tt0)

            _dma_w(*W_CHUNKS[0])
            _dma_x0(*X0_CHUNKS[0])
            _dma_w(*W_CHUNKS[1])
            _dma_x0(*X0_CHUNKS[1])
            _dma_w(*W_CHUNKS[2])
            _dma_x0(*X0_CHUNKS[2])
            _dma_w(*W_CHUNKS[3])
            b_tile = None
            if with_bias:
                b_tile = bpool.tile([P, D], F32, name="b_tile")
                nc.sync.dma_start(b_tile[:], bB[:])

            for ti, (t0, tt) in enumerate(tiles):
                ntc = tt // P  # token chunks in this tile (2 for full tiles)
                last_tile = ti == len(tiles) - 1
                if ti == 0:
                    xv = x0_tiles
                else:
                    xt = xpool.tile([P, NIC * tt], BF16, tag=f"x{ti % 3}")
                    nc.gpsimd.dma_start(xt[:], xP[:, NIC * t0 : NIC * (t0 + tt)])
                    xv = [(xt, ic * tt) for ic in range(NIC)]

                par = ti % 2  # alternate psum bank set
                ps = {
                    (c, ob): psum_pool.tile(
                        [P, FB], F32, name=f"ps_{ti}_{c}_{ob}", tag=f"ps{par}_{c}_{ob}"
                    )
                    for c in range(ntc)
                    for ob in range(NOB)
                }
                ot = opool.tile([P, ntc * D], BF16, name=f"ot_{ti}", tag=f"o{par}")
                for ic in range(NIC):
                    last = ic == NIC - 1
                    for c in range(ntc):
                        xt, xoff = xv[ic]
                        lhsT = xt[:, xoff + c * P : xoff + (c + 1) * P]
                        for ob in range(NOB):
                            wt, woff = w_tiles[ic]
                            nc.tensor.matmul(
                                ps[(c, ob)][:],
                                lhsT,
                                wt[:, woff + ob * FB : woff + (ob + 1) * FB],
                                start=(ic == 0),
                                stop=last,
                            )
                            if last:
                                # evacuate psum as soon as its group closes,
                                # split across two engines
                                dst = ot[:, c * D + ob * FB : c * D + (ob + 1) * FB]
                                if with_bias:
                                    eng = nc.vector if c == 0 else nc.gpsimd
                                    eng.tensor_add(
                                        dst,
                                        ps[(c, ob)][:],
                                        b_tile[:, ob * FB : (ob + 1) * FB],
                                    )
                                elif c == 0:
                                    nc.vector.tensor_copy(dst, ps[(c, ob)][:])
                                else:
                                    nc.scalar.copy(dst, ps[(c, ob)][:])
                g0 = t0 // P
                if last_tile and ntc > 1:
                    # split the final store so its first half overlaps the
                    # trailing casts
                    for c in range(ntc):
                        nc.scalar.dma_start(
                            outP[:, (g0 + c) * D : (g0 + c + 1) * D],
                            ot[:, c * D : (c + 1) * D],
                        )
                else:
                    nc.scalar.dma_start(
                        outP[:, g0 * D : (g0 + ntc) * D], ot[:]
                    )
    nc.compile()
    _program_cache[key] = nc
    return nc


def _pack_x(flat_rows: np.ndarray, C: int) -> np.ndarray:
    """[n, D] fp32 tokens -> [P, NIC*C] bf16 in (tile, ic)-block layout."""
    n = flat_rows.shape[0]
    xT = np.zeros((D, C), dtype=np.float32)
    if n:
        xT[:, :n] = flat_rows.T
    xP = np.empty((P, NIC * C), dtype=BF16NP)
    for t0, tt in _t_tiles(C):
        base = NIC * t0
        for ic in range(NIC):
            xP[:, base + ic * tt : base + (ic + 1) * tt] = xT[
                ic * P : (ic + 1) * P, t0 : t0 + tt
            ].astype(BF16NP)
    return xP


def kernel(actions, action_type, W, b, _trace=False):
    actions = np.ascontiguousarray(actions, dtype=np.float32)
    B, L, _ = actions.shape
    flat = actions.reshape(B * L, D)
    types = np.asarray(action_type).reshape(B * L).astype(np.int64)

    idx = [np.flatnonzero(types == t) for t in range(N_CORES)]
    counts = [len(i) for i in idx]
    # Cap device capacity at 2048 (8 uniform 256-token tiles); rare
    # overflow tokens beyond that are computed on the host instead.
    C = max(P, min(2048, -(-max(counts[1:]) // P) * P))

    W = np.asarray(W, dtype=np.float32)
    b_np = np.asarray(b, dtype=np.float32)

    with_bias = bool(np.any(b_np))
    in_maps = []
    for t in range(N_CORES):
        n_dev = 0 if t == 0 else min(counts[t], C)
        rows = flat[idx[t][:n_dev]] if n_dev else np.zeros((0, D), np.float32)
        wT = np.eye(D, dtype=np.float32) if t == 0 else W[t - 1].T
        wP = np.empty((P, NIC * D), dtype=BF16NP)
        for ic in range(NIC):
            wP[:, ic * D : (ic + 1) * D] = wT[ic * P : (ic + 1) * P, :].astype(BF16NP)
        m = {"xP": _pack_x(rows, C), "wP": wP}
        if with_bias:
            bvec = np.zeros(D, dtype=np.float32) if t == 0 else b_np[t - 1]
            m["bB"] = np.ascontiguousarray(
                np.broadcast_to(bvec, (P, D)), dtype=np.float32
            )
        in_maps.append(m)

    nc = build_program(C, with_bias)
    r = run_bass_kernel_spmd(nc, in_maps, list(range(N_CORES)), trace=_trace)

    out_flat = np.empty_like(flat)
    out_flat[idx[0]] = flat[idx[0]]  # identity tokens: exact copy
    for t in range(1, N_CORES):
        n_dev = min(counts[t], C)
        if n_dev:
            # outP [P, (C//P)*D] -> [C, D]: token g*128+p at [p, g*D:f]
            o = (
                r.results[t]["outP"]
                .reshape(P, C // P, D)
                .transpose(1, 0, 2)
                .reshape(C, D)
            )
            out_flat[idx[t][:n_dev]] = o[:n_dev].astype(np.float32)
        if counts[t] > n_dev:  # overflow beyond device capacity: host BLAS
            ov = idx[t][n_dev:]
            out_flat[ov] = flat[ov] @ W[t - 1].T + b_np[t - 1]
    out = out_flat.reshape(B, L, D)
    if _trace:
        return out, r
    return out
